# revision 8
# baseline (speedup 1.0000x reference)
"""Trainium2 Bass kernel for nn_Classifier_48223892799748 (retrieval_knn).

Computes sim = (D + enc_pm @ cent_pm.T) / 2 where
  enc_pm = sign((samples - 0.5) @ weight.T)  in {+1,-1}
  cent_pm = centroids mapped {0,1} -> {-1,+1}

Sharding: data-parallel over the batch dim (8192 -> 1024 rows per core,
8 cores). weight / centroids replicated.

Device layout: everything is computed transposed ([D, B] / [C, B]) so the
matmul-1 output tile [128 d, 512 b] feeds matmul-2 directly as the moving
operand (contraction over d) with no on-device transpose.

Both matmuls run as fp8e4m3 DoubleRow (256-deep contraction per pass,
2x the f32r/bf16 MAC rate):
  matmul-1: samples quantized to fp8 on host (measured end-to-end rel err
            ~8e-3 from sign flips near the threshold; weights +/-1 are
            exact in fp8, PSUM accumulates in f32 exactly).
  matmul-2: exact (+/-1 x +/-1 or +/-1 x +/-0.5 products, integer-scaled
            f32 accumulation).

Sign encodings are produced per 512-wide batch chunk on two engines in
parallel so neither trails the PE:
  b=0: ACT  Sign        -> {+1,-1}   (ps2[0] = agree,   out = 0.5*ps2 + D/2)
  b=1: DVE  (x>0)-0.5   -> {+.5,-.5} (ps2[1] = agree/2, out = 1.0*ps2 + D/2)
"""

import sys

if "/opt/trn_rl_repo" not in sys.path:
    sys.path.insert(0, "/opt/trn_rl_repo")

import os as _os

import ml_dtypes
import numpy as np

import concourse.bass as bass
import concourse.mybir as mybir
import concourse.tile as tile
from concourse import bacc
from concourse.bass_utils import run_bass_kernel_spmd

# The container's `antenv` package is a stub without `axon_hooks`; if tracing
# is ever requested (BASS_TRACE=1), run_bass_kernel_spmd imports it and would
# crash. Provide a stub module (hook=None -> tracing skipped gracefully)
# unless something (e.g. a test harness) registered a real one already.
try:  # pragma: no cover
    import antenv.axon_hooks  # noqa: F401
except ImportError:
    import types as _types

    import antenv as _antenv

    _hooks = _types.ModuleType("antenv.axon_hooks")
    _hook_store = {"h": None}
    _hooks.set_axon_ntff_profile_hook = lambda h: _hook_store.__setitem__("h", h)
    _hooks.get_axon_ntff_profile_hook = lambda: _hook_store["h"]
    sys.modules["antenv.axon_hooks"] = _hooks
    _antenv.axon_hooks = _hooks

FP8 = ml_dtypes.float8_e4m3

B, IN_F, D, C = 8192, 1024, 10000, 100
N_CORES = 8
B_SH = B // N_CORES          # 1024 batch rows per core
KCP = IN_F // 256            # 4 DoubleRow contraction pairs for matmul 1
DT = (D + 127) // 128        # 79 d-tiles
D_PAD = DT * 128             # 10112
NB = B_SH // 512             # 2 psum-width chunks of the local batch
NPAIR = (DT + 1) // 2        # 40 d-tile pairs for DoubleRow matmul-2
D_PAD2 = NPAIR * 256         # 10240
C_PAD = 112                  # DoubleRow weight AP needs byte-step %16 == 0
CENTER = 0.5

# sign engine: "dve" -> both chunks as (x>0)-0.5 on DVE (+/-0.5 encodings;
#   HW ACT Sign returns NaN for the exact-zero / tiny PSUM values that the
#   fp8 lattice produces, so Sign is avoided entirely);
# "split" -> b=0 ACT Sign, b=1 DVE; "act" -> both on ACT Sign.
SIGN_MODE = _os.environ.get("SIGN_MODE", "dve")

# Stash of the last BassKernelResults (exec_time_ns etc.) for test harnesses.
LAST_RUN = None
_NC_CACHE = None


def _build_nc():
    nc = bacc.Bacc("TRN2", target_bir_lowering=False)
    f32 = mybir.dt.float32
    fp8 = mybir.dt.float8e4
    SIGN = mybir.ActivationFunctionType.Sign
    COPY = mybir.ActivationFunctionType.Copy
    DR = mybir.MatmulPerfMode.DoubleRow

    # DRAM I/O (per-core shard layouts, see host prep in kernel()):
    #   s8:  [128 p, KCP, 2, B_SH] fp8   s8[p,t,j,b] = fp8(samples[b, (2t+j)*128+p] - 0.5)
    #   w8:  [DT, 128 p, KCP, 2, 128 d]  w8[dt,p,t,j,d] = W[dt*128+d, (2t+j)*128+p]
    #   ct:  [128 p, NPAIR, 2, C_PAD]    ct[p,t,j,c] = cent_pm[c, t*256+j*128+p]
    #   out: [C, B_SH] f32               sim.T shard
    s8_d = nc.dram_tensor("s8", [128, KCP, 2, B_SH], fp8, kind="ExternalInput")
    w8_d = nc.dram_tensor("w8", [DT, 128, KCP, 2, 128], fp8, kind="ExternalInput")
    ct_d = nc.dram_tensor("ct", [128, NPAIR, 2, C_PAD], fp8, kind="ExternalInput")
    out_d = nc.dram_tensor("out", [C, B_SH], f32, kind="ExternalOutput")

    with tile.TileContext(nc) as tc:
        with (
            tc.tile_pool(name="const", bufs=1) as const_pool,
            tc.tile_pool(name="wts", bufs=6) as w_pool,
            tc.tile_pool(name="enc", bufs=3) as enc_pool,
            tc.tile_pool(name="outp", bufs=1) as out_pool,
            tc.tile_pool(name="ps1", bufs=3, space=bass.MemorySpace.PSUM) as ps1_pool,
            tc.tile_pool(name="ps2", bufs=1, space=bass.MemorySpace.PSUM) as ps2_pool,
        ):
            s8 = const_pool.tile([128, KCP, 2, B_SH], fp8)
            cent = const_pool.tile([128, NPAIR, 2, C_PAD], fp8)
            # fast start: trigger issue on Sync costs ~650 ns apiece, so
            # order the preamble DMAs by first-need time. The first matmul
            # only needs w00 (32 KB) + the (t=0, b=0) sample chunk (128 KB).
            w00 = const_pool.tile([128, 2, 128], fp8)
            nc.sync.dma_start(w00[:], w8_d[0, :, 0, :, :])
            nc.sync.dma_start(s8[:, 0, :, 0:512], s8_d[:, 0, :, 0:512])
            nc.sync.dma_start(s8[:, 0, :, 512:B_SH], s8_d[:, 0, :, 512:B_SH])

            ps2 = [
                ps2_pool.tile([C_PAD, 512], f32, tag=f"ps2_{b}", name=f"ps2_{b}")
                for b in range(NB)
            ]

            # software pipeline: matmul2 for pair t0 is issued on PE one pair
            # late, so PE never waits on the sign round-trip.
            pending = []

            def flush_pending():
                t0, encs = pending.pop(0)
                for b in range(NB):
                    nc.tensor.matmul(
                        ps2[b][:],
                        cent[:, t0, :, :],
                        encs[b][:],
                        start=(t0 == 0),
                        stop=(t0 == NPAIR - 1),
                        perf_mode=DR,
                    )

            cur_pair = None
            for dt in range(DT):
                w = w_pool.tile([128, KCP, 2, 128], fp8, tag="w", name=f"w_{dt}")
                # one trigger per 128 KB tile: the runtime shards each
                # transfer into ~1.6 KB packets across all 16 DMA engines,
                # so a single trigger still gets full aggregate bandwidth
                if dt == 0:
                    # dt=0 splits by halves interleaved with the sample
                    # loads so everything lands in first-need order
                    nc.sync.dma_start(w[:, : KCP // 2], w8_d[dt, :, : KCP // 2])
                    nc.sync.dma_start(s8[:, 1, :, :], s8_d[:, 1, :, :])
                    nc.sync.dma_start(w[:, KCP // 2 :], w8_d[dt, :, KCP // 2 :])
                    nc.sync.dma_start(s8[:, 2, :, :], s8_d[:, 2, :, :])
                    nc.sync.dma_start(s8[:, 3, :, :], s8_d[:, 3, :, :])
                    nc.sync.dma_start(cent[:], ct_d[:])
                else:
                    nc.sync.dma_start(w[:], w8_d[dt])
                ps1 = [
                    ps1_pool.tile([128, 512], f32, tag=f"ps1_{b}", name=f"ps1_{dt}_{b}")
                    for b in range(NB)
                ]
                for t in range(KCP):
                    w_src = w00[:] if (dt == 0 and t == 0) else w[:, t, :, :]
                    for b in range(NB):
                        nc.tensor.matmul(
                            ps1[b][:],
                            w_src,
                            s8[:, t, :, bass.ts(b, 512)],
                            start=(t == 0),
                            stop=(t == KCP - 1),
                            perf_mode=DR,
                        )
                j = dt % 2
                if j == 0:
                    cur_pair = [
                        enc_pool.tile([128, 2, 512], fp8, tag=f"enc_{b}", name=f"e_{dt}_{b}")
                        for b in range(NB)
                    ]
                for b in range(NB):
                    use_act = SIGN_MODE == "act" or (SIGN_MODE == "split" and b == 0)
                    if use_act:
                        nc.scalar.activation(cur_pair[b][:, j, :], ps1[b][:], SIGN)
                    else:
                        nc.vector.tensor_scalar(
                            out=cur_pair[b][:, j, :],
                            in0=ps1[b][:],
                            scalar1=0.0,
                            scalar2=0.5,
                            op0=mybir.AluOpType.is_gt,
                            op1=mybir.AluOpType.subtract,
                        )
                if dt == DT - 1 and j == 0:
                    # phantom j=1 half of the final pair (dt=79 doesn't
                    # exist): zero it so 0-weight x garbage can't poison PSUM
                    for b in range(NB):
                        nc.gpsimd.memset(cur_pair[b][:, 1, :], 0.0)
                if j == 1 or dt == DT - 1:
                    pending.append((dt // 2, cur_pair))
                if len(pending) >= 2:
                    flush_pending()
            while pending:
                flush_pending()

            for b in range(NB):
                ob = out_pool.tile([C, 512], f32, tag=f"ob_{b}", name=f"ob_{b}")
                use_act = SIGN_MODE == "act" or (SIGN_MODE == "split" and b == 0)
                scale = 0.5 if use_act else 1.0
                nc.scalar.activation(ob[:], ps2[b][:C, :], COPY, bias=D / 2.0, scale=scale)
                nc.sync.dma_start(out_d[:, bass.ts(b, 512)], ob[:])

    nc.compile()
    return nc


def _get_nc():
    global _NC_CACHE
    if _NC_CACHE is None:
        _NC_CACHE = _build_nc()
    return _NC_CACHE


def kernel(samples, weight, centroids):
    global LAST_RUN
    samples = np.asarray(samples, dtype=np.float32)
    weight = np.asarray(weight, dtype=np.float32)
    centroids = np.asarray(centroids)

    # ---- host-side marshalling (layout + dtype only) ----
    # centered samples, transposed to [IN_F, B], quantized to fp8e4m3
    scT8 = (samples - np.float32(CENTER)).T.astype(FP8)

    def s_core(c):
        # [IN_F, B_SH] -> [128 p, KCP, 2, B_SH]
        blk = scT8[:, c * B_SH : (c + 1) * B_SH]
        return np.ascontiguousarray(
            blk.reshape(KCP, 2, 128, B_SH).transpose(2, 0, 1, 3)
        )

    # weight.T DoubleRow tiles: w8[dt, p, t, j, d] = W[dt*128+d, (2t+j)*128+p]
    wpad = np.zeros((D_PAD, IN_F), dtype=np.float32)
    wpad[:D] = weight  # +/-1, exact in fp8
    w8 = np.ascontiguousarray(
        wpad.reshape(DT, 128, KCP, 2, 128).transpose(0, 4, 2, 3, 1).astype(FP8)
    )

    # DoubleRow centroid tiles: ct[p, t, j, c] = cent_pm[c, t*256+j*128+p]
    cpad = np.zeros((D_PAD2, C_PAD), dtype=np.float32)
    cpad[:D, :C] = np.where(centroids, np.float32(1.0), np.float32(-1.0)).T
    ct = np.ascontiguousarray(
        cpad.reshape(NPAIR, 2, 128, C_PAD).transpose(2, 0, 1, 3).astype(FP8)
    )

    in_maps = [{"s8": s_core(c), "w8": w8, "ct": ct} for c in range(N_CORES)]

    nc = _get_nc()
    res = run_bass_kernel_spmd(nc, in_maps, core_ids=list(range(N_CORES)))
    LAST_RUN = res

    # gather: out[c] is sim.T for batch rows [c*B_SH, (c+1)*B_SH)
    return np.vstack(
        [np.asarray(res.results[c]["out"]).T for c in range(N_CORES)]
    ).astype(np.float32)
